# revision 1
# baseline (speedup 1.0000x reference)
"""Multi-head attention (B=4, N=1024, C=1024, H=16) on 8 TRN2 NeuronCores.

Sharding: core c handles batch b = c//2 and query-row half g = c%2.
Data parallel over B; within a batch pair, the V projection is tensor
parallel over heads: each core computes V only for its 8 heads (g picks
the W_v column half on the host), then a pair AllGather assembles the
full V in global head order on both cores.  The V exchange is the only
collective: it is needed late (first woven PV) so its ~55us end-to-end
latency hides under the K/Q projections.  K is computed redundantly on
both cores of a pair — a K exchange was measured to always expose
20-30us of collective latency before the first attention round, more
than the 17us of matmul it saves.  Q is computed locally for the core's
own 512 query rows over all 16 heads, each core runs full attention +
output projection for its 512 rows, and output rows are disjoint so no
collective is needed after the projection.

Compute is bf16 on the TensorEngine with fp32 PSUM accumulation; softmax
is computed without max-subtraction (logits are bounded ~2.5 for this
problem) as exp(S^T)@[V*e, e] with V as the stationary operand; the
denominator lands as an extra PSUM row, is bounced to SBUF partition 0
(the approx-reciprocal DVE op misreads PSUM at base partition 64) for a
fast approximate reciprocal on VectorE, broadcast back across partitions
on GpSimd, and multiplied in on VectorE.  e = exp(-5*(1-mask)) folds the
additive mask penalty in exactly.

Schedule: V chunks project first and upload so the gather flies during
the K/Q projections; the gather downloads DMA strided straight into V_s
(no engine copies — the Tile scheduler would hoist them ahead of work
the first round needs).  Attention rounds are ScalarE(exp)-bound; q
chunks 2,3 backfill TensorE inside the first rounds.  The tail runs
pair 7's PV as soon as its exps land and overlaps the softmax normalize
chain with three projection-prefix accumulations (the third borrowing
the PSUM banks pair 6's PV freed), so only the fc=7 closes trail the
last PT chunk.  Output is stored bf16 (rel-err budget has >6x headroom).
"""

import numpy as np
import ml_dtypes

import concourse.bass as bass
import concourse.mybir as mybir
import concourse.tile as tile
from concourse import bacc

N_CORES = 8
B, N, C = 4, 1024, 1024
H = 16
D = C // H  # 64
NQ = N // 2  # query rows per core: 512
P = 128
KC = C // P  # 8 contraction chunks
KH = 4  # own-half feature chunks (8 heads = 512 features)
SCALE = D ** -0.5
PAIR_GROUPS = [[0, 1], [2, 3], [4, 5], [6, 7]]

F32 = mybir.dt.float32
BF16 = mybir.dt.bfloat16
AF = mybir.ActivationFunctionType


def build_nc():
    nc = bacc.Bacc(None, num_devices=N_CORES)

    xT = nc.declare_dram_parameter("xT", [C, N], BF16, isOutput=False)
    xqT = nc.declare_dram_parameter("xqT", [C, NQ], BF16, isOutput=False)
    W_q = nc.declare_dram_parameter("W_q", [C, C], BF16, isOutput=False)
    W_k = nc.declare_dram_parameter("W_k", [C, C], BF16, isOutput=False)
    W_v = nc.declare_dram_parameter("W_v", [C, C // 2], BF16, isOutput=False)
    W_p = nc.declare_dram_parameter("W_p", [C, C], BF16, isOutput=False)
    e_in = nc.declare_dram_parameter("e", [N], F32, isOutput=False)
    b_in = nc.declare_dram_parameter("b", [C], F32, isOutput=False)
    out_ext = nc.declare_dram_parameter("out", [NQ, C], BF16, isOutput=True)

    with tile.TileContext(nc) as tc, (
        tc.tile_pool(name="acts", bufs=1)
    ) as apool, (
        tc.tile_pool(name="early", bufs=1)
    ) as early, (
        tc.tile_pool(name="work", bufs=2)
    ) as work, (
        tc.tile_pool(name="chain", bufs=2)
    ) as chain, (
        tc.tile_pool(name="dram", bufs=1, space="DRAM")
    ) as dram, (
        tc.tile_pool(name="ps_big", bufs=2, space="PSUM")
    ) as ps_big, (
        tc.tile_pool(name="ps_pv", bufs=2, space="PSUM")
    ) as ps_pv:
        e_s = apool.tile([P, KC], F32)
        nc.sync.dma_start(e_s[:], e_in.rearrange("(o p) -> p o", p=P))

        # ---- PE warmup while the loads stream in --------------------
        warm = early.tile([P, 512], BF16, tag="warm")
        nc.vector.memset(warm[:], 0.0)
        wps = ps_big.tile([P, 1024], F32, tag="big")
        for i in range(6):
            nc.tensor.matmul(wps[:, 0:512], warm[:, 0:P], warm[:],
                             start=True, stop=True)

        # ---- static loads (chunked so matmuls can start early) ------
        Wk_s = early.tile([P, KC, C], BF16)
        xT_s = early.tile([P, KC, N], BF16)
        Wq_s = early.tile([P, KC, C], BF16)
        Wv_s = early.tile([P, KC, C // 2], BF16)
        xqT_s = early.tile([P, KC, NQ], BF16)
        for kc in range(KC):
            nc.sync.dma_start(xT_s[:, kc, :], xT[kc * P:(kc + 1) * P, :])
            nc.sync.dma_start(Wv_s[:, kc, :], W_v[kc * P:(kc + 1) * P, :])
        Wp_s = apool.tile([P, KC, C], BF16)
        bias_s = apool.tile([P, C], F32)

        # V_own stages the core's own V half for the gather upload; the
        # downloads then DMA both halves strided straight into V_s in
        # global head order (no engine copies on the critical path).
        KT_s = apool.tile([P, KC, N], BF16)
        QT_s = apool.tile([P, KC, NQ], BF16)
        V_own = apool.tile([P, KC, 8, D], BF16)
        V_s = apool.tile([P, KC, H, D + 1], BF16)
        PT_s = apool.tile([P, KC, NQ], BF16)
        for mc in range(KC):
            nc.vector.tensor_copy(
                V_s[:, mc, :, D:D + 1],
                e_s[:, mc:mc + 1, None].to_broadcast((P, H, 1)),
            )

        # ---- DRAM bounce buffers + pair AllGather -------------------
        in_v = dram.tile([P, KC, 8, D], BF16)
        out_v = dram.tile([2 * P, KC, 8, D], BF16)

        def k_chunk(fc):
            """K^T feature chunk fc, all 16 heads (computed redundantly on
            both cores of the pair — the K-gather latency can't hide)."""
            ps = ps_big.tile([P, 1024], F32, tag="big", name="ps_k")
            for nh in range(2):
                for kc in range(KC):
                    nc.tensor.matmul(
                        ps[:, nh * 512:(nh + 1) * 512],
                        Wk_s[:, kc, fc * P:(fc + 1) * P],
                        xT_s[:, kc, nh * 512:(nh + 1) * 512],
                        start=(kc == 0),
                        stop=(kc == KC - 1),
                    )
            nc.vector.tensor_copy(KT_s[:, fc, :], ps[:])

        def q_chunk(fc2):
            ps = ps_big.tile([P, 1024], F32, tag="big", name="ps_q")
            for half in range(2):
                fc = 2 * fc2 + half
                for kc in range(KC):
                    nc.tensor.matmul(
                        ps[:, half * 512:(half + 1) * 512],
                        Wq_s[:, kc, fc * P:(fc + 1) * P],
                        xqT_s[:, kc, :],
                        start=(kc == 0),
                        stop=(kc == KC - 1),
                    )
            nc.vector.tensor_copy(
                QT_s[:, 2 * fc2:2 * fc2 + 2, :],
                ps[:].rearrange("p (a b) -> p a b", a=2),
            )

        def v_chunk(mc):
            """Own-half V (8 heads) for key chunk mc, scaled by e."""
            psf = ps_big.tile([P, 1024], F32, tag="big", name="ps_v")
            ps = psf[:, 0:512]
            for kc in range(KC):
                nc.tensor.matmul(
                    ps[:],
                    xT_s[:, kc, mc * P:(mc + 1) * P],
                    Wv_s[:, kc, :],
                    start=(kc == 0),
                    stop=(kc == KC - 1),
                )
            nc.vector.tensor_scalar_mul(
                V_own[:, mc, :, :],
                ps[:].rearrange("p (h d) -> p h d", d=D),
                e_s[:, mc:mc + 1],
            )

        def st_exp(hp, expT, pv_hp=None, pv_expT=None, pv=None,
                   pv_points={1: [0], 3: [1]}):
            """S^T+exp tiles for pair hp.  When a previous pair's PV is
            supplied, its matmuls are emitted in two 8-matmul chunks
            between S^T groups: ScalarE keeps a 2-tile exp backlog to
            drain while TensorE runs the PV chunk, and the V stationary
            operand only swaps twice per round (LDWEIGHTS stays
            pipelined)."""
            for kcp in range(4):
                for h01 in range(2):
                    lo, hi = h01 * 64, h01 * 64 + 64
                    ps = ps_big.tile([P, 1024], F32, tag="big", name="ps_st")
                    for j in range(2):
                        kc = 2 * kcp + j
                        nc.tensor.matmul(
                            ps[:, j * 512:(j + 1) * 512],
                            KT_s[lo:hi, hp, kc * P:(kc + 1) * P],
                            QT_s[lo:hi, hp, :],
                            start=True,
                            stop=True,
                        )
                    nc.scalar.activation(
                        expT[h01][:, 2 * kcp:2 * kcp + 2, :],
                        ps[:].rearrange("p (a b) -> p a b", a=2),
                        AF.Exp,
                        scale=SCALE,
                    )
                if pv is not None and kcp in pv_points:
                    for grp in pv_points[kcp]:
                        for h01 in range(2):
                            for kc in range(4 * grp, 4 * grp + 4):
                                nc.tensor.matmul(
                                    pv[0:D + 1, h01, :],
                                    V_s[:, kc, 2 * pv_hp + h01, :],
                                    pv_expT[h01][:, kc, :],
                                    start=(kc == 0),
                                    stop=(kc == KC - 1),
                                )

        def pv_norm(hp, expT, pv=None):
            if pv is None:
                pv = ps_pv.tile([P, 2, NQ], F32, tag="pv")
                for h01 in range(2):
                    h = 2 * hp + h01
                    for kc in range(KC):
                        nc.tensor.matmul(
                            pv[0:D + 1, h01, :],
                            V_s[:, kc, h, :],
                            expT[h01][:, kc, :],
                            start=(kc == 0),
                            stop=(kc == KC - 1),
                        )
            # reciprocal_approx_fast's bitwise-NOT DVE op misreads PSUM at
            # base partition 64 (verified on hw) — bounce the denominator
            # row through SBUF partition 0 first.
            den = chain.tile([1, 2, NQ], F32, tag="den", bufs=1)
            rcr = chain.tile([1, 2, NQ], F32, tag="rcr", bufs=1)
            nc.vector.tensor_copy(den[0:1], pv[D:D + 1, :, :])
            nc.vector.reciprocal_approx_fast(rcr[0:1], den[0:1])
            for h01 in range(2):
                bcast = chain.tile([D, NQ], F32, tag=f"bcast{h01}",
                                   name=f"bcast{h01}")
                nc.gpsimd.partition_broadcast(bcast[:], rcr[0:1, h01, :])
                nc.vector.tensor_mul(
                    PT_s[h01 * D:(h01 + 1) * D, hp, :],
                    pv[0:D, h01, :],
                    bcast[:],
                )

        # ---- K/V projection + exchange, Q projection ----------------
        def gather(in_t, out_t):
            nc.gpsimd.collective_compute(
                "AllGather",
                mybir.AluOpType.bypass,
                replica_groups=PAIR_GROUPS,
                ins=[in_t[:].opt()],
                outs=[out_t[:].opt()],
            )

        # V projects first (two mc-halves) so its gathers fly while K/Q
        # project; only the 3MB of xT/Wv loads sit ahead of the V uploads
        # in the sync DMA queue, so the exchange completes ~50us before
        # the first woven PV needs it.  Wk loads issue between the two
        # uploads — in time for k_chunk(0) but not delaying the V path.
        for mc in range(4):
            v_chunk(mc)
        nc.sync.dma_start(in_v[:, 0:4], V_own[:, 0:4])
        for kc in range(KC):
            nc.sync.dma_start(Wk_s[:, kc, :], W_k[kc * P:(kc + 1) * P, :])
        for mc in range(4, KC):
            v_chunk(mc)
        nc.sync.dma_start(in_v[:, 4:KC], V_own[:, 4:KC])
        gather(in_v, out_v)
        for kc in range(KC):
            nc.sync.dma_start(xqT_s[:, kc, :], xqT[kc * P:(kc + 1) * P, :])
            nc.sync.dma_start(Wq_s[:, kc, :], W_q[kc * P:(kc + 1) * P, :])

        for fc in range(KC):
            k_chunk(fc)
        q_chunk(0)
        q_chunk(1)
        for mc in range(KC):
            nc.sync.dma_start(V_s[:, mc, 0:8, 0:D], out_v[0:P, mc])
            nc.sync.dma_start(V_s[:, mc, 8:16, 0:D], out_v[P:2 * P, mc])
        nc.sync.dma_start(Wp_s[:], W_p.rearrange("(ko p) n -> p ko n", p=P))
        nc.sync.dma_start(bias_s[:], b_in[None, :].to_broadcast((P, C)))

        # ---- software-pipelined attention rounds --------------------
        # q chunks 2,3 are woven between the first rounds: the exp stream
        # (the ScalarE bottleneck) starts as soon as q0/q1 land while
        # TensorE back-fills the remaining Q projection.
        expTs = {}

        def new_expT(i):
            return [
                work.tile([P, KC, NQ], BF16, tag=f"exp{i % 2}_{h01}",
                          name=f"expT{h01}", bufs=1)
                for h01 in range(2)
            ]

        expTs[0] = new_expT(0)
        st_exp(0, expTs[0])
        q_chunk(2)
        for i in range(1, KC):
            expTs[i] = new_expT(i)
            pv = ps_pv.tile([P, 2, NQ], F32, tag="pv", name="pv")
            # Rounds 1-2's PV waits until after their last S^T group:
            # pairs 0-1's V halves arrive from the gather only just in
            # time.
            pts = {3: [0, 1]} if i in (1, 2) else {1: [0], 3: [1]}
            st_exp(i, expTs[i], i - 1, expTs[i - 1], pv, pv_points=pts)
            pv_norm(i - 1, expTs[i - 1], pv)
            if i == 1:
                q_chunk(3)

        # ---- output projection + bias -------------------------------
        # First two query chunks accumulate feature chunks 0..6 early so
        # TensorE stays busy while ScalarE drains the last pair's exps;
        # the fc=7 matmuls land after pv_norm(7) writes PT chunk 7.
        def proj_accum(ps, qs, fcs, start):
            for nn in range(2):
                for fc in fcs:
                    nc.tensor.matmul(
                        ps[:, nn * 512:(nn + 1) * 512],
                        PT_s[:, fc, qs * P:(qs + 1) * P],
                        Wp_s[:, fc, nn * 512:(nn + 1) * 512],
                        start=(start and fc == fcs[0]),
                        stop=(fc == KC - 1),
                    )

        def proj_store(ps, qs):
            o_sb = work.tile([P, 1024], BF16, tag="osb")
            nc.vector.tensor_add(o_sb[:], ps[:], bias_s[:])
            nc.sync.dma_start(out_ext[qs * P:(qs + 1) * P, :], o_sb[:])

        # Pair 7's PV fires as soon as its exps land; its normalize chain
        # (vector/gpsimd) overlaps the three projection prefixes, the
        # third of which borrows the ps_pv bank pair freed by pair 6.
        pv7 = ps_pv.tile([P, 2, NQ], F32, tag="pv", name="pv")
        for h01 in range(2):
            for kc in range(KC):
                nc.tensor.matmul(
                    pv7[0:D + 1, h01, :],
                    V_s[:, kc, 2 * (KC - 1) + h01, :],
                    expTs[KC - 1][h01][:, kc, :],
                    start=(kc == 0),
                    stop=(kc == KC - 1),
                )
        ps_pj = {}
        for qs in range(2):
            ps_pj[qs] = ps_big.tile([P, 1024], F32, tag="big",
                                    name=f"ps_pj{qs}")
            proj_accum(ps_pj[qs], qs, list(range(KC - 1)), start=True)
        pv_norm(KC - 1, expTs[KC - 1], pv7)
        ps_pj[2] = ps_pv.tile([P, 2, NQ], F32, tag="pv",
                              name="ps_pj2").rearrange("p a b -> p (a b)")
        proj_accum(ps_pj[2], 2, list(range(KC - 1)), start=True)
        for qs in range(3):
            proj_accum(ps_pj[qs], qs, [KC - 1], start=False)
            proj_store(ps_pj[qs], qs)
        ps3 = ps_big.tile([P, 1024], F32, tag="big", name="ps_pj3")
        proj_accum(ps3, 3, list(range(KC)), start=True)
        proj_store(ps3, 3)

    nc.finalize()
    return nc


def make_in_maps(x, mask, W_qkv, W_proj, b_proj):
    bf = ml_dtypes.bfloat16
    x = np.asarray(x, np.float32)
    mask = np.asarray(mask, np.float32)
    W_qkv = np.asarray(W_qkv, np.float32)
    W_proj = np.asarray(W_proj, np.float32)
    b_proj = np.asarray(b_proj, np.float32)

    W_q = np.ascontiguousarray(W_qkv[:, 0:C]).astype(bf)
    W_k_full = np.ascontiguousarray(W_qkv[:, C:2 * C]).astype(bf)
    W_p = np.ascontiguousarray(W_proj).astype(bf)
    e_all = np.exp(-5.0 * (1.0 - mask)).astype(np.float32)  # [B, N]

    in_maps = []
    for c in range(N_CORES):
        b, g = divmod(c, 2)
        xT = np.ascontiguousarray(x[b].T).astype(bf)
        xqT = np.ascontiguousarray(x[b, g * NQ:(g + 1) * NQ, :].T).astype(bf)
        W_k = W_k_full
        W_v = np.ascontiguousarray(
            W_qkv[:, 2 * C + g * 512:2 * C + (g + 1) * 512]).astype(bf)
        in_maps.append({
            "xT": xT, "xqT": xqT, "W_q": W_q, "W_k": W_k, "W_v": W_v,
            "W_p": W_p, "e": np.ascontiguousarray(e_all[b]),
            "b": b_proj,
        })
    return in_maps


def assemble_output(results):
    out = np.zeros((B, N, C), np.float32)
    for c in range(N_CORES):
        b, g = divmod(c, 2)
        out[b, g * NQ:(g + 1) * NQ, :] = np.asarray(
            results[c]["out"], np.float32)
    return out


def kernel(x, mask, W_qkv, W_proj, b_proj):
    from concourse.bass_utils import run_bass_kernel_spmd

    nc = build_nc()
    in_maps = make_in_maps(x, mask, W_qkv, W_proj, b_proj)
    res = run_bass_kernel_spmd(nc, in_maps, core_ids=list(range(N_CORES)))
    return assemble_output(res.results)



# revision 8
# speedup vs baseline: 1.0396x; 1.0396x over previous
"""Multi-head attention (B=4, N=1024, C=1024, H=16) on 8 TRN2 NeuronCores.

Sharding: core c handles batch b = c//2 and query-row half g = c%2.
Data parallel over B; within a batch pair, the V projection is tensor
parallel over heads: each core computes V only for its 8 heads (g picks
the W_v column half on the host), then a pair AllGather assembles the
full V in global head order on both cores.  The V exchange is the only
collective: it is needed late (first woven PV) so its ~55us end-to-end
latency hides under the K/Q projections.  K is computed redundantly on
both cores of a pair — a K exchange was measured to always expose
20-30us of collective latency before the first attention round, more
than the 17us of matmul it saves.  Q is computed locally for the core's
own 512 query rows over all 16 heads, each core runs full attention +
output projection for its 512 rows, and output rows are disjoint so no
collective is needed after the projection.

Compute is bf16 on the TensorEngine with fp32 PSUM accumulation; softmax
is computed without max-subtraction (logits are bounded ~2.5 for this
problem) as exp(S^T)@[V*e, e] with V as the stationary operand; the
denominator lands as an extra PSUM row, is bounced to SBUF partition 0
(the approx-reciprocal DVE op misreads PSUM at base partition 64) for a
fast approximate reciprocal on VectorE, broadcast back across partitions
on GpSimd, and multiplied in on VectorE.  e = exp(-5*(1-mask)) folds the
additive mask penalty in exactly.

Schedule: V chunks project first and upload so the gather flies during
the K/Q projections; the gather downloads DMA strided straight into V_s
(no engine copies — the Tile scheduler would hoist them ahead of work
the first round needs).  Attention rounds are ScalarE(exp)-bound; q
chunks 2,3 backfill TensorE inside the first rounds.  The tail runs
pair 7's PV as soon as its exps land and overlaps the softmax normalize
chain with three projection-prefix accumulations (the third borrowing
the PSUM banks pair 6's PV freed), so only the fc=7 closes trail the
last PT chunk.  Output is stored bf16 (rel-err budget has >6x headroom).
"""

import numpy as np
import ml_dtypes

import concourse.bass as bass
import concourse.mybir as mybir
import concourse.tile as tile
from concourse import bacc

N_CORES = 8
B, N, C = 4, 1024, 1024
H = 16
D = C // H  # 64
NQ = N // 2  # query rows per core: 512
P = 128
KC = C // P  # 8 contraction chunks
KH = 4  # own-half feature chunks (8 heads = 512 features)
SCALE = D ** -0.5
PAIR_GROUPS = [[0, 1], [2, 3], [4, 5], [6, 7]]

F32 = mybir.dt.float32
BF16 = mybir.dt.bfloat16
AF = mybir.ActivationFunctionType


def build_nc():
    nc = bacc.Bacc(None, num_devices=N_CORES)

    xT = nc.declare_dram_parameter("xT", [C, N], BF16, isOutput=False)
    xqT = nc.declare_dram_parameter("xqT", [C, NQ], BF16, isOutput=False)
    W_q = nc.declare_dram_parameter("W_q", [C, C], BF16, isOutput=False)
    W_k = nc.declare_dram_parameter("W_k", [C, C], BF16, isOutput=False)
    W_v = nc.declare_dram_parameter("W_v", [C, C // 2], BF16, isOutput=False)
    W_p = nc.declare_dram_parameter("W_p", [C, C], BF16, isOutput=False)
    e_in = nc.declare_dram_parameter("e", [N], F32, isOutput=False)
    b_in = nc.declare_dram_parameter("b", [C], F32, isOutput=False)
    out_ext = nc.declare_dram_parameter("out", [NQ, C], BF16, isOutput=True)

    with tile.TileContext(nc) as tc, (
        tc.tile_pool(name="acts", bufs=1)
    ) as apool, (
        tc.tile_pool(name="early", bufs=1)
    ) as early, (
        tc.tile_pool(name="work", bufs=2)
    ) as work, (
        tc.tile_pool(name="chain", bufs=2)
    ) as chain, (
        tc.tile_pool(name="dram", bufs=1, space="DRAM")
    ) as dram, (
        tc.tile_pool(name="ps_big", bufs=2, space="PSUM")
    ) as ps_big, (
        tc.tile_pool(name="ps_pv", bufs=2, space="PSUM")
    ) as ps_pv:
        e_s = apool.tile([P, KC], F32)
        nc.sync.dma_start(e_s[:], e_in.rearrange("(o p) -> p o", p=P))

        # ---- PE warmup while the loads stream in --------------------
        warm = early.tile([P, 512], BF16, tag="warm")
        nc.vector.memset(warm[:], 0.0)
        wps = ps_big.tile([P, 1024], F32, tag="big")
        for i in range(6):
            nc.tensor.matmul(wps[:, 0:512], warm[:, 0:P], warm[:],
                             start=True, stop=True)

        # ---- static loads (chunked so matmuls can start early) ------
        Wk_s = early.tile([P, KC, C], BF16)
        xT_s = early.tile([P, KC, N], BF16)
        Wq_s = early.tile([P, KC, C], BF16)
        Wv_s = early.tile([P, KC, C // 2], BF16)
        xqT_s = early.tile([P, KC, NQ], BF16)
        for kc in range(KC):
            nc.sync.dma_start(xT_s[:, kc, :], xT[kc * P:(kc + 1) * P, :])
            nc.sync.dma_start(Wv_s[:, kc, :], W_v[kc * P:(kc + 1) * P, :])
        Wp_s = apool.tile([P, KC, C], BF16)
        bias_s = apool.tile([P, C], F32)

        # V_own stages the core's own V half for the gather upload; the
        # downloads then DMA both halves strided straight into V_s in
        # global head order (no engine copies on the critical path).
        # V_s is flat so each head's PV stationary can be read as a
        # 128-column window [V_h | e | spill-into-next-head]: a 128-col
        # stationary enables Fast Weight Load, which hides LDWEIGHTS
        # under the previous matmul's stream (433ns -> 216ns per PV
        # matmul measured).  The 63-col tail pad makes the last head's
        # window stay in bounds; its contents are never read as results
        # (PV output rows 65-127 are discarded).
        KT_s = apool.tile([P, KC, N], BF16)
        QT_s = apool.tile([P, KC, NQ], BF16)
        V_own = apool.tile([P, KC, 8, D], BF16)
        V_f = apool.tile([P, KC * H * (D + 1) + 63], BF16)
        V_s = V_f[:, 0:KC * H * (D + 1)].rearrange(
            "p (k h c) -> p k h c", h=H, c=D + 1)
        PT_s = apool.tile([P, KC, NQ], BF16)
        for mc in range(KC):
            nc.vector.tensor_copy(
                V_s[:, mc, :, D:D + 1],
                e_s[:, mc:mc + 1, None].to_broadcast((P, H, 1)),
            )

        def v_stat(kc, h):
            """128-col PV stationary window for head h, key chunk kc."""
            s = (kc * H + h) * (D + 1)
            return V_f[:, s:s + 128]

        # ---- DRAM bounce buffers + pair AllGather -------------------
        in_v = dram.tile([P, KC, 8, D], BF16)
        out_v = dram.tile([2 * P, KC, 8, D], BF16)
        dum_i = dram.tile([1, 16], BF16)
        dum_o = dram.tile([2, 16], BF16)

        def k_chunk(fc):
            """K^T feature chunk fc, all 16 heads (computed redundantly on
            both cores of the pair — the K-gather latency can't hide)."""
            ps = ps_big.tile([P, 1024], F32, tag="big", name="ps_k")
            for nh in range(2):
                for kc in range(KC):
                    nc.tensor.matmul(
                        ps[:, nh * 512:(nh + 1) * 512],
                        Wk_s[:, kc, fc * P:(fc + 1) * P],
                        xT_s[:, kc, nh * 512:(nh + 1) * 512],
                        start=(kc == 0),
                        stop=(kc == KC - 1),
                    )
            nc.vector.tensor_copy(KT_s[:, fc, :], ps[:])

        def q_chunk(fc2):
            ps = ps_big.tile([P, 1024], F32, tag="big", name="ps_q")
            for half in range(2):
                fc = 2 * fc2 + half
                for kc in range(KC):
                    nc.tensor.matmul(
                        ps[:, half * 512:(half + 1) * 512],
                        Wq_s[:, kc, fc * P:(fc + 1) * P],
                        xqT_s[:, kc, :],
                        start=(kc == 0),
                        stop=(kc == KC - 1),
                    )
            nc.vector.tensor_copy(
                QT_s[:, 2 * fc2:2 * fc2 + 2, :],
                ps[:].rearrange("p (a b) -> p a b", a=2),
            )

        def v_chunk(mc):
            """Own-half V (8 heads) for key chunk mc, scaled by e."""
            psf = ps_big.tile([P, 1024], F32, tag="big", name="ps_v")
            ps = psf[:, 0:512]
            for kc in range(KC):
                nc.tensor.matmul(
                    ps[:],
                    xT_s[:, kc, mc * P:(mc + 1) * P],
                    Wv_s[:, kc, :],
                    start=(kc == 0),
                    stop=(kc == KC - 1),
                )
            nc.vector.tensor_scalar_mul(
                V_own[:, mc, :, :],
                ps[:].rearrange("p (h d) -> p h d", d=D),
                e_s[:, mc:mc + 1],
            )

        def st_exp(hp, expT, pv_hp=None, pv_expT=None, pv=None,
                   pv_points={1: [0], 3: [1]}):
            """S^T+exp tiles for pair hp.  When a previous pair's PV is
            supplied, its matmuls are emitted in two 8-matmul chunks
            between S^T groups: ScalarE keeps a 2-tile exp backlog to
            drain while TensorE runs the PV chunk, and the V stationary
            operand only swaps twice per round (LDWEIGHTS stays
            pipelined)."""
            for kcp in range(4):
                for h01 in range(2):
                    lo, hi = h01 * 64, h01 * 64 + 64
                    ps = ps_big.tile([P, 1024], F32, tag="big", name="ps_st")
                    for j in range(2):
                        kc = 2 * kcp + j
                        nc.tensor.matmul(
                            ps[:, j * 512:(j + 1) * 512],
                            KT_s[lo:hi, hp, kc * P:(kc + 1) * P],
                            QT_s[lo:hi, hp, :],
                            start=True,
                            stop=True,
                        )
                    nc.scalar.activation(
                        expT[h01][:, 2 * kcp:2 * kcp + 2, :],
                        ps[:].rearrange("p (a b) -> p a b", a=2),
                        AF.Exp,
                        scale=SCALE,
                    )
                if pv is not None and kcp in pv_points:
                    for grp in pv_points[kcp]:
                        for h01 in range(2):
                            for kc in range(4 * grp, 4 * grp + 4):
                                nc.tensor.matmul(
                                    pv[:, h01, :],
                                    v_stat(kc, 2 * pv_hp + h01),
                                    pv_expT[h01][:, kc, :],
                                    start=(kc == 0),
                                    stop=(kc == KC - 1),
                                )

        def pv_norm(hp, expT, pv=None):
            if pv is None:
                pv = ps_pv.tile([P, 2, NQ], F32, tag="pv")
                for h01 in range(2):
                    h = 2 * hp + h01
                    for kc in range(KC):
                        nc.tensor.matmul(
                            pv[:, h01, :],
                            v_stat(kc, h),
                            expT[h01][:, kc, :],
                            start=(kc == 0),
                            stop=(kc == KC - 1),
                        )
            # reciprocal_approx_fast's bitwise-NOT DVE op misreads PSUM at
            # base partition 64 (verified on hw) — bounce the denominator
            # row through SBUF partition 0 first.
            den = chain.tile([1, 2, NQ], F32, tag="den", bufs=1)
            rcr = chain.tile([1, 2, NQ], F32, tag="rcr", bufs=1)
            nc.vector.tensor_copy(den[0:1], pv[D:D + 1, :, :])
            nc.vector.reciprocal_approx_fast(rcr[0:1], den[0:1])
            for h01 in range(2):
                bcast = chain.tile([D, NQ], F32, tag=f"bcast{h01}",
                                   name=f"bcast{h01}")
                nc.gpsimd.partition_broadcast(bcast[:], rcr[0:1, h01, :])
                nc.vector.tensor_mul(
                    PT_s[h01 * D:(h01 + 1) * D, hp, :],
                    pv[0:D, h01, :],
                    bcast[:],
                )

        # ---- K/V projection + exchange, Q projection ----------------
        def gather(in_t, out_t):
            nc.gpsimd.collective_compute(
                "AllGather",
                mybir.AluOpType.bypass,
                replica_groups=PAIR_GROUPS,
                ins=[in_t[:].opt()],
                outs=[out_t[:].opt()],
            )

        # A dependency-free dummy gather issues first: the first CC op
        # pays a ~30us cross-core rendezvous (start skew), so absorb it
        # while the input DMAs stream instead of ahead of the real
        # exchange.
        gather(dum_i, dum_o)

        # V projects first (two mc-halves) so its gathers fly while K/Q
        # project; only the 3MB of xT/Wv loads sit ahead of the V uploads
        # in the sync DMA queue.  Wk loads issue between the two
        # uploads — in time for k_chunk(0) but not delaying the V path.
        for mc in range(4):
            v_chunk(mc)
        nc.sync.dma_start(in_v[:, 0:4], V_own[:, 0:4])
        for kc in range(4):
            nc.sync.dma_start(Wk_s[:, kc, :], W_k[kc * P:(kc + 1) * P, :])
        for mc in range(4, KC):
            v_chunk(mc)
        nc.sync.dma_start(in_v[:, 4:KC], V_own[:, 4:KC])
        gather(in_v, out_v)
        for kc in range(4, KC):
            nc.sync.dma_start(Wk_s[:, kc, :], W_k[kc * P:(kc + 1) * P, :])
        for kc in range(KC):
            nc.sync.dma_start(xqT_s[:, kc, :], xqT[kc * P:(kc + 1) * P, :])
            nc.sync.dma_start(Wq_s[:, kc, :], W_q[kc * P:(kc + 1) * P, :])

        # Gather downloads ride the GpSimd DMA queue so they fire the
        # moment the exchange lands instead of queueing behind the
        # xqT/Wq/Wp loads on the sync queue.
        for mc in range(KC):
            nc.gpsimd.dma_start(V_s[:, mc, 0:8, 0:D], out_v[0:P, mc])
            nc.gpsimd.dma_start(V_s[:, mc, 8:16, 0:D], out_v[P:2 * P, mc])

        for fc in range(KC):
            k_chunk(fc)
        q_chunk(0)
        q_chunk(1)
        nc.sync.dma_start(Wp_s[:], W_p.rearrange("(ko p) n -> p ko n", p=P))
        nc.sync.dma_start(bias_s[:], b_in[None, :].to_broadcast((P, C)))

        # ---- software-pipelined attention rounds --------------------
        # q chunks 2,3 are woven between the first rounds: the exp stream
        # (the ScalarE bottleneck) starts as soon as q0/q1 land while
        # TensorE back-fills the remaining Q projection.
        expTs = {}

        def new_expT(i):
            return [
                work.tile([P, KC, NQ], BF16, tag=f"exp{i % 2}_{h01}",
                          name=f"expT{h01}", bufs=1)
                for h01 in range(2)
            ]

        expTs[0] = new_expT(0)
        st_exp(0, expTs[0])
        q_chunk(2)
        for i in range(1, KC):
            expTs[i] = new_expT(i)
            pv = ps_pv.tile([P, 2, NQ], F32, tag="pv", name="pv")
            # Rounds 1-2's PV waits until after their last S^T group:
            # pairs 0-1's V halves arrive from the gather only just in
            # time.
            pts = {3: [0, 1]} if i in (1, 2) else {1: [0], 3: [1]}
            st_exp(i, expTs[i], i - 1, expTs[i - 1], pv, pv_points=pts)
            pv_norm(i - 1, expTs[i - 1], pv)
            if i == 1:
                q_chunk(3)

        # ---- output projection + bias -------------------------------
        # First two query chunks accumulate feature chunks 0..6 early so
        # TensorE stays busy while ScalarE drains the last pair's exps;
        # the fc=7 matmuls land after pv_norm(7) writes PT chunk 7.
        def proj_accum(ps, qs, fcs, start):
            for nn in range(2):
                for fc in fcs:
                    nc.tensor.matmul(
                        ps[:, nn * 512:(nn + 1) * 512],
                        PT_s[:, fc, qs * P:(qs + 1) * P],
                        Wp_s[:, fc, nn * 512:(nn + 1) * 512],
                        start=(start and fc == fcs[0]),
                        stop=(fc == KC - 1),
                    )

        def proj_store(ps, qs):
            o_sb = work.tile([P, 1024], BF16, tag="osb")
            nc.vector.tensor_add(o_sb[:], ps[:], bias_s[:])
            nc.sync.dma_start(out_ext[qs * P:(qs + 1) * P, :], o_sb[:])

        # Pair 7's PV fires as soon as its exps land; its normalize chain
        # (vector/gpsimd) overlaps the three projection prefixes, the
        # third of which borrows the ps_pv bank pair freed by pair 6.
        pv7 = ps_pv.tile([P, 2, NQ], F32, tag="pv", name="pv")
        for h01 in range(2):
            for kc in range(KC):
                nc.tensor.matmul(
                    pv7[:, h01, :],
                    v_stat(kc, 2 * (KC - 1) + h01),
                    expTs[KC - 1][h01][:, kc, :],
                    start=(kc == 0),
                    stop=(kc == KC - 1),
                )
        ps_pj = {}
        for qs in range(2):
            ps_pj[qs] = ps_big.tile([P, 1024], F32, tag="big",
                                    name=f"ps_pj{qs}")
            proj_accum(ps_pj[qs], qs, list(range(KC - 1)), start=True)
        pv_norm(KC - 1, expTs[KC - 1], pv7)
        ps_pj[2] = ps_pv.tile([P, 2, NQ], F32, tag="pv",
                              name="ps_pj2").rearrange("p a b -> p (a b)")
        proj_accum(ps_pj[2], 2, list(range(KC - 1)), start=True)
        for qs in range(3):
            proj_accum(ps_pj[qs], qs, [KC - 1], start=False)
            proj_store(ps_pj[qs], qs)
        ps3 = ps_big.tile([P, 1024], F32, tag="big", name="ps_pj3")
        proj_accum(ps3, 3, list(range(KC)), start=True)
        proj_store(ps3, 3)

    nc.finalize()
    return nc


def make_in_maps(x, mask, W_qkv, W_proj, b_proj):
    bf = ml_dtypes.bfloat16
    x = np.asarray(x, np.float32)
    mask = np.asarray(mask, np.float32)
    W_qkv = np.asarray(W_qkv, np.float32)
    W_proj = np.asarray(W_proj, np.float32)
    b_proj = np.asarray(b_proj, np.float32)

    W_q = np.ascontiguousarray(W_qkv[:, 0:C]).astype(bf)
    W_k_full = np.ascontiguousarray(W_qkv[:, C:2 * C]).astype(bf)
    W_p = np.ascontiguousarray(W_proj).astype(bf)
    e_all = np.exp(-5.0 * (1.0 - mask)).astype(np.float32)  # [B, N]

    in_maps = []
    for c in range(N_CORES):
        b, g = divmod(c, 2)
        xT = np.ascontiguousarray(x[b].T).astype(bf)
        xqT = np.ascontiguousarray(x[b, g * NQ:(g + 1) * NQ, :].T).astype(bf)
        W_k = W_k_full
        W_v = np.ascontiguousarray(
            W_qkv[:, 2 * C + g * 512:2 * C + (g + 1) * 512]).astype(bf)
        in_maps.append({
            "xT": xT, "xqT": xqT, "W_q": W_q, "W_k": W_k, "W_v": W_v,
            "W_p": W_p, "e": np.ascontiguousarray(e_all[b]),
            "b": b_proj,
        })
    return in_maps


def assemble_output(results):
    out = np.zeros((B, N, C), np.float32)
    for c in range(N_CORES):
        b, g = divmod(c, 2)
        out[b, g * NQ:(g + 1) * NQ, :] = np.asarray(
            results[c]["out"], np.float32)
    return out


def kernel(x, mask, W_qkv, W_proj, b_proj):
    from concourse.bass_utils import run_bass_kernel_spmd

    nc = build_nc()
    in_maps = make_in_maps(x, mask, W_qkv, W_proj, b_proj)
    res = run_bass_kernel_spmd(nc, in_maps, core_ids=list(range(N_CORES)))
    return assemble_output(res.results)

